# revision 5
# baseline (speedup 1.0000x reference)
"""HTATeacher forward on 8 Trainium2 cores (Bass/Tile), self-contained.

Sharding: nodes row-sharded (384/core), hyperedges row-sharded (192/core).
adj mask computed once (bf16 matmuls). Node softmax without max-subtraction
(scores bounded ~17); edge softmax with two-pass max subtraction. Attention
numerators (E) and V in bf16 for the attn@V matmuls; everything else fp32.
AllGathers exchange h_e, h_e_out and layer outputs between cores.

PSUM budget (8 banks): "ps_sc" [128,2048] x1 = 4 banks (attention scores),
"ps_acc" [128,512] x4 = 4 banks (accumulators + all small psum users).
"""
import math
import numpy as np
import ml_dtypes

import concourse.bass as bass
import concourse.tile as tile
from concourse import bacc, mybir
from concourse.bass_utils import run_bass_kernel_spmd
from concourse.masks import make_identity

N, M, F = 3072, 1536, 1433
FP = 1536          # padded feature dim
D0 = 512
HEADS = 8
RANK = 32
NCLS = 7
C = 8
NSH = N // C       # 384
MSH = M // C       # 192
LAYERS = [(512, 256), (256, 128), (128, 128), (128, 128)]

F32 = mybir.dt.float32
BF16 = mybir.dt.bfloat16
AX = mybir.AxisListType.X
ALU = mybir.AluOpType
ACTF = mybir.ActivationFunctionType
ECH = [(0, 128), (128, 64)]   # e-shard chunks


def T(t, k, w):
    return t[:, k * w:(k + 1) * w]


def build_program():
    nc = bacc.Bacc("TRN2", target_bir_lowering=False, debug=False,
                   enable_asserts=True, num_devices=C)

    I = {}
    def inp(name, shape, dt=F32):
        I[name] = nc.dram_tensor(name, shape, dt, kind="ExternalInput").ap()
        return I[name]

    inp("xT", [FP, NSH]); inp("HT_bf", [FP, N], BF16); inp("HTs", [M, NSH])
    inp("Hs", [N, MSH]); inp("Hrows", [NSH, M]); inp("U_rTs", [RANK, NSH])
    inp("inw", [FP, D0]); inp("inb", [1, D0])
    for l, (di, do) in enumerate(LAYERS):
        hd = do // HEADS
        W = 8 * (hd + 1)
        inp(f"Wqp{l}", [di, 256]); inp(f"Wkp{l}", [di, 256]); inp(f"Wva{l}", [di, W])
        inp(f"Wg3{l}", [256, 24]); inp(f"bg3{l}", [1, 24]); inp(f"Wsp{l}", [RANK, do])
        inp(f"Wo{l}", [do, do]); inp(f"bo{l}", [1, do])
        inp(f"lng{l}", [1, do]); inp(f"lnb{l}", [1, do])
    inp("cls_w", [128, NCLS]); inp("cls_b", [1, NCLS])

    out_logits = nc.dram_tensor("logits_sh", [NSH, NCLS], F32, kind="ExternalOutput").ap()
    out_Hd = nc.dram_tensor("Hd_sh", [NSH, M], F32, kind="ExternalOutput").ap()
    out_dm = nc.dram_tensor("dmask", [1, M], F32, kind="ExternalOutput").ap()

    RG = [list(range(C))]

    with tile.TileContext(nc) as tc:
        with (
            tc.tile_pool(name="persist", bufs=1) as pp,
            tc.tile_pool(name="ps", bufs=1, space="PSUM") as ps,
            tc.tile_pool(name="dram", bufs=1, space="DRAM") as dr,
        ):
            ident = pp.tile([128, 128], F32, tag="ident")
            make_identity(nc, ident[:])
            ones_row = pp.tile([1, 128], F32, tag="ones_row")
            nc.gpsimd.memset(ones_row[:], 1.0)
            ones_col = pp.tile([128, 1], F32, tag="ones_col")
            nc.gpsimd.memset(ones_col[:], 1.0)

            # ---------- helpers ----------
            def pacc(shape=None):
                return ps.tile(shape or [128, 512], F32, tag="ps_acc", bufs=4, name="ps_acc")

            def psc_tile():
                return ps.tile([128, 2048], F32, tag="ps_sc", bufs=1, name="ps_sc")

            def bcast_row(row_ap, w, tag):
                """[1, w] -> [128, w] sbuf tile (rows replicated)."""
                t = pp.tile([1, w], F32, tag=tag + "_r", name=tag + "_r")
                nc.sync.dma_start(t[:], row_ap)
                p = pacc()
                nc.tensor.matmul(p[:, :w], ones_row[:], t[:], start=True, stop=True)
                o = pp.tile([128, w], F32, tag=tag, name=tag)
                nc.scalar.activation(o[:], p[:, :w], ACTF.Copy)
                return o

            def transpose_to(dst_ap, src_ap, pn, fn):
                """dst[fn, pn] = src[pn, fn].T via PE; pn, fn <= 128."""
                p = pacc([128, 128])
                nc.tensor.transpose(p[:fn, :pn], src_ap, ident[:pn, :pn])
                nc.vector.tensor_copy(dst_ap, p[:fn, :pn])

            # ---------- persistent loads ----------
            HTs_t = pp.tile([128, 12 * NSH], F32, tag="HTs")
            for kb in range(12):
                nc.sync.dma_start(T(HTs_t, kb, NSH), I["HTs"][kb * 128:(kb + 1) * 128, :])
            Hs_t = pp.tile([128, 24 * MSH], F32, tag="Hs")
            for jb in range(24):
                nc.sync.dma_start(T(Hs_t, jb, MSH), I["Hs"][jb * 128:(jb + 1) * 128, :])
            UrT_t = pp.tile([RANK, NSH], F32, tag="UrT")
            nc.sync.dma_start(UrT_t[:], I["U_rTs"][:])
            clsw_t = pp.tile([128, NCLS], F32, tag="clsw")
            nc.sync.dma_start(clsw_t[:], I["cls_w"][:])

            mask_t = pp.tile([128, 24 * NSH], BF16, tag="mask")

            # ---------- adjacency mask (once) ----------
            with tc.tile_pool(name="adjp", bufs=1) as ap_:
                HTs_bf = ap_.tile([128, 12 * NSH], BF16, tag="HTs_bf")
                nc.vector.tensor_copy(HTs_bf[:], HTs_t[:])
                for jb in range(24):
                    pa = pacc([128, NSH])
                    for kb in range(12):
                        ht = ap_.tile([128, 128], BF16, tag="ht_bf", bufs=3)
                        nc.sync.dma_start(ht[:], I["HT_bf"][kb * 128:(kb + 1) * 128,
                                                            jb * 128:(jb + 1) * 128])
                        nc.tensor.matmul(pa[:], ht[:], T(HTs_bf, kb, NSH),
                                         start=(kb == 0), stop=(kb == 11))
                    nc.vector.tensor_scalar(T(mask_t, jb, NSH), pa[:], 0.0, None,
                                            op0=ALU.is_gt)

            # ---------- input projection ----------
            h_full = [dr.tile([N, D0], F32, tag="h_full0", name="h_full0")]
            hsh_t = pp.tile([128, 3 * 512], F32, tag="hsh")     # layer out shard (natural)
            with tc.tile_pool(name="projp", bufs=1) as jp:
                xT_t = jp.tile([128, 12 * NSH], F32, tag="xT")
                for kb in range(12):
                    nc.sync.dma_start(T(xT_t, kb, NSH), I["xT"][kb * 128:(kb + 1) * 128, :])
                inw_t = jp.tile([128, 12 * D0], F32, tag="inw")
                for kb in range(12):
                    nc.sync.dma_start(T(inw_t, kb, D0), I["inw"][kb * 128:(kb + 1) * 128, :])
                inb_bc = bcast_row(I["inb"][:], D0, "inb_bc")
                h0_b = dr.tile([NSH, D0], F32, tag="h0_b")
                for ic in range(3):
                    p = pacc()
                    for kb in range(12):
                        nc.tensor.matmul(p[:, :D0], xT_t[:, kb * NSH + ic * 128:
                                                         kb * NSH + ic * 128 + 128],
                                         T(inw_t, kb, D0), start=(kb == 0), stop=(kb == 11))
                    nc.vector.tensor_tensor(T(hsh_t, ic, 512)[:, :D0], p[:, :D0],
                                            inb_bc[:], op=ALU.add)
                    nc.sync.dma_start(h0_b[ic * 128:(ic + 1) * 128, :],
                                      T(hsh_t, ic, 512)[:, :D0])
                nc.gpsimd.collective_compute("AllGather", ALU.bypass, replica_groups=RG,
                                             ins=[h0_b.opt()], outs=[h_full[0].opt()])

            heo_full_t = None
            hTsh_t = None

            # ================= layers =================
            for l, (di, do) in enumerate(LAYERS):
                hd = do // HEADS
                s = hd + 1
                W = 8 * s
                kbs = di // 128
                dbs = do // 128
                scale = 1.0 / math.sqrt(hd)
                hf = h_full[l]

                # --- weights/biases ---
                with tc.tile_pool(name=f"lw{l}", bufs=1) as lw:
                    wqp_t = lw.tile([128, kbs * 256], F32, tag="wqp")
                    wkp_t = lw.tile([128, kbs * 256], F32, tag="wkp")
                    wva_t = lw.tile([128, kbs * W], F32, tag="wva")
                    for kb in range(kbs):
                        nc.sync.dma_start(T(wqp_t, kb, 256),
                                          I[f"Wqp{l}"][kb * 128:(kb + 1) * 128, :])
                        nc.sync.dma_start(T(wkp_t, kb, 256),
                                          I[f"Wkp{l}"][kb * 128:(kb + 1) * 128, :])
                        nc.sync.dma_start(T(wva_t, kb, W),
                                          I[f"Wva{l}"][kb * 128:(kb + 1) * 128, :])
                    wg3_t = lw.tile([128, 2 * 24], F32, tag="wg3")
                    for g in range(2):
                        nc.sync.dma_start(T(wg3_t, g, 24),
                                          I[f"Wg3{l}"][g * 128:(g + 1) * 128, :])
                    wsp_t = lw.tile([RANK, do], F32, tag="wsp")
                    nc.sync.dma_start(wsp_t[:], I[f"Wsp{l}"][:])
                    wo_t = lw.tile([128, dbs * do], F32, tag="wo")
                    for db in range(dbs):
                        nc.sync.dma_start(T(wo_t, db, do),
                                          I[f"Wo{l}"][db * 128:(db + 1) * 128, :])
                    bg3_bc = bcast_row(I[f"bg3{l}"][:], 24, "bg3_bc")
                    bo_bc = bcast_row(I[f"bo{l}"][:], do, "bo_bc")
                    lng_bc = bcast_row(I[f"lng{l}"][:], do, "lng_bc")
                    lnb_bc = bcast_row(I[f"lnb{l}"][:], do, "lnb_bc")

                    # --- hTsh: transpose of our shard h (for Q/gates) ---
                    hTsh_t = pp.tile([128, 4 * NSH], F32, tag="hTsh")
                    for ic in range(3):
                        for fb in range(kbs):
                            transpose_to(
                                hTsh_t[:, fb * NSH + ic * 128:fb * NSH + ic * 128 + 128],
                                T(hsh_t, ic, 512)[:, fb * 128:(fb + 1) * 128], 128, 128)

                    # --- QTp [2][128, NSH] ---
                    qTp_t = pp.tile([128, 2 * NSH], F32, tag="qTp")
                    for g in range(2):
                        p = pacc()
                        for kb in range(kbs):
                            nc.tensor.matmul(
                                p[:, :NSH],
                                wqp_t[:, kb * 256 + g * 128:kb * 256 + g * 128 + 128],
                                T(hTsh_t, kb, NSH), start=(kb == 0), stop=(kb == kbs - 1))
                        nc.scalar.activation(T(qTp_t, g, NSH), p[:, :NSH], ACTF.Copy)

                    # --- hT full (transposes of h_full), then KTp / V_aug ---
                    kTp_t = pp.tile([128, 2 * N], F32, tag="kTp")
                    va_t = pp.tile([128, 24 * W], BF16, tag="va")
                    HN = N // 2
                    for hlf in range(2):
                        with tc.tile_pool(name=f"hT{l}_{hlf}", bufs=1) as hp:
                            hT_t = hp.tile([128, kbs * HN], F32, tag="hT")
                            for jb2 in range(12):
                                jb = hlf * 12 + jb2
                                for fb in range(kbs):
                                    hl = hp.tile([128, 128], F32, tag="hload", bufs=3)
                                    nc.sync.dma_start(hl[:], hf[jb * 128:(jb + 1) * 128,
                                                                fb * 128:(fb + 1) * 128])
                                    transpose_to(
                                        hT_t[:, fb * HN + jb2 * 128:fb * HN + jb2 * 128 + 128],
                                        hl[:], 128, 128)
                            for g in range(2):
                                for sl2 in range(3):
                                    sl = hlf * 3 + sl2
                                    p = pacc()
                                    for kb in range(kbs):
                                        nc.tensor.matmul(
                                            p[:, :512],
                                            wkp_t[:, kb * 256 + g * 128:kb * 256 + g * 128 + 128],
                                            hT_t[:, kb * HN + sl2 * 512:kb * HN + sl2 * 512 + 512],
                                            start=(kb == 0), stop=(kb == kbs - 1))
                                    nc.scalar.activation(
                                        T(kTp_t, g, N)[:, sl * 512:(sl + 1) * 512],
                                        p[:, :512], ACTF.Copy)
                            for jb2 in range(12):
                                jb = hlf * 12 + jb2
                                p = pacc([128, W])
                                for kb in range(kbs):
                                    nc.tensor.matmul(
                                        p[:], hT_t[:, kb * HN + jb2 * 128:kb * HN + jb2 * 128 + 128],
                                        T(wva_t, kb, W), start=(kb == 0), stop=(kb == kbs - 1))
                                nc.scalar.activation(T(va_t, jb, W), p[:], ACTF.Copy)
                                nc.gpsimd.memset(T(va_t, jb, W)[:, hd::s], 1.0)

                    # --- edge branch: h_e shard ---
                    he_b = dr.tile([MSH, di], F32, tag=f"he_b{l}")
                    he_f = dr.tile([M, di], F32, tag=f"he_f{l}")
                    nsl = max(1, di // 512)
                    slw = min(512, di)
                    with tc.tile_pool(name=f"ed{l}", bufs=1) as ep:
                        for (e0, esz) in ECH:
                            for fs in range(nsl):
                                p = pacc()
                                for jb in range(24):
                                    hn = ep.tile([128, 512], F32, tag="hnat", bufs=4)
                                    nc.sync.dma_start(hn[:, :slw],
                                                      hf[jb * 128:(jb + 1) * 128,
                                                         fs * 512:fs * 512 + slw])
                                    nc.tensor.matmul(p[:esz, :slw],
                                                     T(Hs_t, jb, MSH)[:, e0:e0 + esz],
                                                     hn[:, :slw], start=(jb == 0),
                                                     stop=(jb == 23))
                                hesb = ep.tile([128, 512], F32, tag="hesb", bufs=2)
                                nc.vector.tensor_copy(hesb[:esz, :slw], p[:esz, :slw])
                                nc.sync.dma_start(he_b[e0:e0 + esz, fs * 512:fs * 512 + slw],
                                                  hesb[:esz, :slw])
                        nc.gpsimd.collective_compute("AllGather", ALU.bypass,
                                                     replica_groups=RG,
                                                     ins=[he_b.opt()], outs=[he_f.opt()])

                    # --- heT (full) + heT_sh (local) + edge QKV ---
                    qeTp_t = pp.tile([128, 2 * MSH], F32, tag="qeTp")
                    keTp_t = pp.tile([128, 2 * M], F32, tag="keTp")
                    vea_t = pp.tile([128, 12 * W], BF16, tag="vea")
                    with tc.tile_pool(name=f"het{l}", bufs=1) as tp_:
                        heT_t = tp_.tile([128, kbs * M], F32, tag="heT")
                        for kb in range(12):
                            for fb in range(kbs):
                                hl2 = tp_.tile([128, 128], F32, tag="hel", bufs=4)
                                nc.sync.dma_start(hl2[:], he_f[kb * 128:(kb + 1) * 128,
                                                               fb * 128:(fb + 1) * 128])
                                transpose_to(
                                    heT_t[:, fb * M + kb * 128:fb * M + kb * 128 + 128],
                                    hl2[:], 128, 128)
                        heTsh_t = tp_.tile([128, 4 * MSH], F32, tag="heTsh")
                        for (e0, esz) in ECH:
                            for fb in range(kbs):
                                hl3 = tp_.tile([128, 128], F32, tag="hel2", bufs=4)
                                nc.sync.dma_start(hl3[:esz, :],
                                                  he_b[e0:e0 + esz, fb * 128:(fb + 1) * 128])
                                transpose_to(
                                    heTsh_t[:, fb * MSH + e0:fb * MSH + e0 + esz],
                                    hl3[:esz, :], esz, 128)
                        for g in range(2):
                            p = pacc()
                            for kb in range(kbs):
                                nc.tensor.matmul(
                                    p[:, :MSH],
                                    wqp_t[:, kb * 256 + g * 128:kb * 256 + g * 128 + 128],
                                    T(heTsh_t, kb, MSH), start=(kb == 0), stop=(kb == kbs - 1))
                            nc.scalar.activation(T(qeTp_t, g, MSH), p[:, :MSH], ACTF.Copy)
                            for sl in range(3):
                                p2 = pacc()
                                for kb in range(kbs):
                                    nc.tensor.matmul(
                                        p2[:, :512],
                                        wkp_t[:, kb * 256 + g * 128:kb * 256 + g * 128 + 128],
                                        heT_t[:, kb * M + sl * 512:kb * M + sl * 512 + 512],
                                        start=(kb == 0), stop=(kb == kbs - 1))
                                nc.scalar.activation(
                                    T(keTp_t, g, M)[:, sl * 512:(sl + 1) * 512],
                                    p2[:, :512], ACTF.Copy)
                        for kb in range(12):
                            p = pacc([128, W])
                            for fb in range(kbs):
                                nc.tensor.matmul(
                                    p[:], heT_t[:, fb * M + kb * 128:fb * M + kb * 128 + 128],
                                    T(wva_t, fb, W), start=(fb == 0), stop=(fb == kbs - 1))
                            nc.scalar.activation(T(vea_t, kb, W), p[:], ACTF.Copy)
                            nc.gpsimd.memset(T(vea_t, kb, W)[:, hd::s], 1.0)

                    # --- edge attention (two-pass: max, then exp) ---
                    heo_b = dr.tile([MSH, do], F32, tag=f"heo_b{l}")
                    heo_f = dr.tile([M, do], F32, tag=f"heo_f{l}")
                    with tc.tile_pool(name=f"ea{l}", bufs=1) as ea:
                        pheo = [pacc([128, W]) for _ in range(2)]
                        for g in range(2):
                            rmax = [ea.tile([128, MSH], F32, tag=f"rmax{r}", name=f"rmax{r}") for r in range(4)]
                            for kb in range(12):
                                psc = psc_tile()
                                for r in range(4):
                                    nc.tensor.matmul(psc[:, 512 * r:512 * r + MSH],
                                                     T(keTp_t, g, M)[32 * r:32 * r + 32,
                                                                     kb * 128:kb * 128 + 128],
                                                     T(qeTp_t, g, MSH)[32 * r:32 * r + 32, :],
                                                     start=True, stop=True,
                                                     tile_position=(32 * r, 0))
                                for r in range(4):
                                    if kb == 0:
                                        nc.vector.tensor_copy(rmax[r][:],
                                                              psc[:, 512 * r:512 * r + MSH])
                                    else:
                                        nc.vector.tensor_tensor(rmax[r][:], rmax[r][:],
                                                                psc[:, 512 * r:512 * r + MSH],
                                                                op=ALU.max)
                            for r in range(4):
                                h = 4 * g + r
                                nmT = ea.tile([1, MSH], F32, tag="nmT", bufs=2)
                                for (e0, esz) in ECH:
                                    ptr = pacc([128, 128])
                                    nc.tensor.transpose(ptr[:esz, :128],
                                                        rmax[r][:, e0:e0 + esz], ident[:])
                                    cm = ea.tile([128, 1], F32, tag="cm", bufs=2)
                                    nc.vector.tensor_reduce(cm[:esz, :], ptr[:esz, :128],
                                                            axis=AX, op=ALU.max)
                                    ptr2 = pacc([128, 128])
                                    nc.tensor.transpose(ptr2[:1, :esz], cm[:esz, :],
                                                        ident[:esz, :esz])
                                    nc.vector.tensor_scalar(nmT[:, e0:e0 + esz],
                                                            ptr2[:1, :esz],
                                                            -1.0, None, op0=ALU.mult)
                                pbc = pacc([128, MSH])
                                nc.tensor.matmul(pbc[:], ones_row[:], nmT[:],
                                                 start=True, stop=True)
                                nmbc = ea.tile([128, MSH], F32, tag="nmbc", bufs=2)
                                nc.scalar.activation(nmbc[:], pbc[:], ACTF.Copy)
                                ebf = ea.tile([128, 12 * MSH], BF16, tag="ebf", bufs=2)
                                for kb in range(12):
                                    psc2 = psc_tile()
                                    nc.tensor.matmul(psc2[:, 512 * r:512 * r + MSH],
                                                     T(keTp_t, g, M)[32 * r:32 * r + 32,
                                                                     kb * 128:kb * 128 + 128],
                                                     T(qeTp_t, g, MSH)[32 * r:32 * r + 32, :],
                                                     start=True, stop=True,
                                                     tile_position=(32 * r, 0))
                                    esub = ea.tile([128, MSH], F32, tag="esub", bufs=3)
                                    nc.vector.tensor_tensor(esub[:],
                                                            psc2[:, 512 * r:512 * r + MSH],
                                                            nmbc[:], op=ALU.add)
                                    nc.scalar.activation(T(ebf, kb, MSH), esub[:], ACTF.Exp,
                                                         scale=scale)
                                    for ci, (e0, esz) in enumerate(ECH):
                                        nc.tensor.matmul(pheo[ci][:esz, h * s:h * s + s],
                                                         T(ebf, kb, MSH)[:, e0:e0 + esz],
                                                         T(vea_t, kb, W)[:, h * s:h * s + s],
                                                         start=(kb == 0), stop=(kb == 11),
                                                         skip_group_check=True)
                        for ci, (e0, esz) in enumerate(ECH):
                            rec = ea.tile([128, 8], F32, tag="rec", bufs=2)
                            nc.vector.reciprocal(rec[:esz, :], pheo[ci][:esz, hd::s])
                            heo_sb = ea.tile([128, do], F32, tag="heo_sb", bufs=2)
                            for h in range(HEADS):
                                nc.vector.tensor_scalar(heo_sb[:esz, h * hd:(h + 1) * hd],
                                                        pheo[ci][:esz, h * s:h * s + hd],
                                                        rec[:esz, h:h + 1], None, op0=ALU.mult)
                            nc.sync.dma_start(heo_b[e0:e0 + esz, :], heo_sb[:esz, :])
                        nc.gpsimd.collective_compute("AllGather", ALU.bypass,
                                                     replica_groups=RG,
                                                     ins=[heo_b.opt()], outs=[heo_f.opt()])
                    heo_full_t = pp.tile([128, 12 * 256], F32, tag="heo_full")
                    for kb in range(12):
                        nc.sync.dma_start(T(heo_full_t, kb, 256)[:, :do],
                                          heo_f[kb * 128:(kb + 1) * 128, :])

                    # --- node attention ---
                    pon = [pacc([128, W]) for _ in range(3)]
                    with tc.tile_pool(name=f"na{l}", bufs=1) as na:
                        for jb in range(24):
                            for g in range(2):
                                psc = psc_tile()
                                for r in range(4):
                                    nc.tensor.matmul(psc[:, 512 * r:512 * r + NSH],
                                                     T(kTp_t, g, N)[32 * r:32 * r + 32,
                                                                    jb * 128:jb * 128 + 128],
                                                     T(qTp_t, g, NSH)[32 * r:32 * r + 32, :],
                                                     start=True, stop=True,
                                                     tile_position=(32 * r, 0))
                                egr = na.tile([128, 4 * NSH], BF16, tag="egr", bufs=3)
                                for r in range(4):
                                    nc.scalar.activation(T(egr, r, NSH),
                                                         psc[:, 512 * r:512 * r + NSH],
                                                         ACTF.Exp, scale=scale)
                                    nc.vector.tensor_tensor(T(egr, r, NSH), T(egr, r, NSH),
                                                            T(mask_t, jb, NSH), op=ALU.mult)
                                    h = 4 * g + r
                                    for ic in range(3):
                                        nc.tensor.matmul(
                                            pon[ic][:, h * s:h * s + s],
                                            T(egr, r, NSH)[:, ic * 128:ic * 128 + 128],
                                            T(va_t, jb, W)[:, h * s:h * s + s],
                                            start=(jb == 0), stop=(jb == 23),
                                            skip_group_check=True)
                        onode = [None] * 3
                        for ic in range(3):
                            rec = na.tile([128, 8], F32, tag="recn", bufs=2)
                            nc.vector.reciprocal(rec[:], pon[ic][:, hd::s])
                            onode[ic] = na.tile([128, do], F32, tag=f"onode{ic}", name=f"onode{ic}")
                            for h in range(HEADS):
                                nc.vector.tensor_scalar(onode[ic][:, h * hd:(h + 1) * hd],
                                                        pon[ic][:, h * s:h * s + hd],
                                                        rec[:, h:h + 1], None, op0=ALU.mult)

                        # --- gates / spec / edge-out / combine / Wout / LN ---
                        hout_b = dr.tile([NSH, do], F32, tag=f"hout_b{l}")
                        gT_t = na.tile([128, dbs * NSH], F32, tag="gT")
                        for ic in range(3):
                            pg = pacc([128, 24])
                            for g in range(2):
                                nc.tensor.matmul(
                                    pg[:], T(qTp_t, g, NSH)[:, ic * 128:ic * 128 + 128],
                                    T(wg3_t, g, 24), start=(g == 0), stop=(g == 1))
                            sg = na.tile([128, 24], F32, tag="sg", bufs=2)
                            nc.vector.tensor_tensor(sg[:], pg[:], bg3_bc[:], op=ALU.add)
                            nc.scalar.activation(sg[:], sg[:], ACTF.Sigmoid)
                            gm = na.tile([128, 3], F32, tag="gm", bufs=2)
                            nc.vector.tensor_reduce(gm[:],
                                                    sg[:].rearrange("p (g k) -> p g k", k=8),
                                                    axis=AX, op=ALU.add)
                            t1 = na.tile([128, 8], F32, tag="gtmp", bufs=2)
                            # cols: 0 sum8(gn)+sum8(ge), 1 gs, 2 total, 3 rec, 4 rec/8,
                            #       5 wn, 6 we, 7 ws
                            nc.vector.tensor_tensor(t1[:, 0:1], gm[:, 0:1], gm[:, 1:2],
                                                    op=ALU.add)
                            nc.vector.tensor_scalar(t1[:, 1:2], t1[:, 0:1], -0.125, 1.0,
                                                    op0=ALU.mult, op1=ALU.add)
                            nc.vector.tensor_scalar(t1[:, 1:2], t1[:, 1:2], 0.0, None,
                                                    op0=ALU.max)
                            nc.vector.tensor_scalar(t1[:, 2:3], t1[:, 0:1], 0.125, 1e-8,
                                                    op0=ALU.mult, op1=ALU.add)
                            nc.vector.tensor_tensor(t1[:, 2:3], t1[:, 2:3], t1[:, 1:2],
                                                    op=ALU.add)
                            nc.vector.reciprocal(t1[:, 3:4], t1[:, 2:3])
                            nc.vector.tensor_scalar(t1[:, 4:5], t1[:, 3:4], 0.125, None,
                                                    op0=ALU.mult)
                            nc.vector.tensor_tensor(t1[:, 5:6], gm[:, 0:1], t1[:, 4:5],
                                                    op=ALU.mult)
                            nc.vector.tensor_tensor(t1[:, 6:7], gm[:, 1:2], t1[:, 4:5],
                                                    op=ALU.mult)
                            nc.vector.tensor_tensor(t1[:, 7:8], t1[:, 1:2], t1[:, 3:4],
                                                    op=ALU.mult)
                            # fold spec_gate = mean(sigmoid(Q@Wgs+bgs)) into ws
                            nc.vector.tensor_tensor(t1[:, 7:8], t1[:, 7:8], gm[:, 2:3],
                                                    op=ALU.mult)
                            nc.vector.tensor_scalar(t1[:, 7:8], t1[:, 7:8], 0.125, None,
                                                    op0=ALU.mult)

                            po = pacc([128, 256])
                            for kb in range(12):
                                nc.tensor.matmul(po[:, :do],
                                                 T(HTs_t, kb, NSH)[:, ic * 128:ic * 128 + 128],
                                                 T(heo_full_t, kb, 256)[:, :do],
                                                 start=(kb == 0), stop=(kb == 11))
                            psp = pacc([128, 256])
                            nc.tensor.matmul(psp[:, :do], UrT_t[:, ic * 128:ic * 128 + 128],
                                             wsp_t[:], start=True, stop=True)

                            comb = na.tile([128, do], F32, tag="comb", bufs=2)
                            nc.vector.tensor_scalar(comb[:], onode[ic][:], t1[:, 5:6], None,
                                                    op0=ALU.mult)
                            tmp = na.tile([128, do], F32, tag="ctmp", bufs=2)
                            nc.vector.tensor_scalar(tmp[:], po[:, :do], t1[:, 6:7], None,
                                                    op0=ALU.mult)
                            nc.vector.tensor_tensor(comb[:], comb[:], tmp[:], op=ALU.add)
                            nc.vector.tensor_scalar(tmp[:], psp[:, :do], t1[:, 7:8], None,
                                                    op0=ALU.mult)
                            nc.vector.tensor_tensor(comb[:], comb[:], tmp[:], op=ALU.add)
                            for fb in range(dbs):
                                transpose_to(
                                    gT_t[:, fb * NSH + ic * 128:fb * NSH + ic * 128 + 128],
                                    comb[:, fb * 128:(fb + 1) * 128], 128, 128)

                        for ic in range(3):
                            p2 = pacc([128, 256])
                            for fb in range(dbs):
                                nc.tensor.matmul(
                                    p2[:, :do],
                                    gT_t[:, fb * NSH + ic * 128:fb * NSH + ic * 128 + 128],
                                    T(wo_t, fb, do), start=(fb == 0), stop=(fb == dbs - 1))
                            z = na.tile([128, do], F32, tag="z", bufs=2)
                            nc.vector.tensor_tensor(z[:], p2[:, :do], bo_bc[:], op=ALU.add)
                            if di == do:
                                nc.vector.tensor_tensor(z[:], z[:],
                                                        T(hsh_t, ic, 512)[:, :do], op=ALU.add)
                            else:
                                nc.vector.tensor_scalar(z[:], z[:], 2.0, None, op0=ALU.mult)
                            mu = na.tile([128, 4], F32, tag="mu", bufs=2)
                            nc.vector.tensor_reduce(mu[:, 0:1], z[:], axis=AX, op=ALU.add)
                            nc.vector.tensor_scalar(mu[:, 0:1], mu[:, 0:1], 1.0 / do, None,
                                                    op0=ALU.mult)
                            nc.vector.tensor_scalar(z[:], z[:], mu[:, 0:1], None,
                                                    op0=ALU.subtract)
                            sq = na.tile([128, do], F32, tag="sq", bufs=2)
                            nc.scalar.activation(sq[:], z[:], ACTF.Square,
                                                 accum_out=mu[:, 1:2])
                            nc.vector.tensor_scalar(mu[:, 1:2], mu[:, 1:2], 1.0 / do, 1e-5,
                                                    op0=ALU.mult, op1=ALU.add)
                            nc.scalar.activation(mu[:, 2:3], mu[:, 1:2], ACTF.Sqrt)
                            nc.vector.reciprocal(mu[:, 3:4], mu[:, 2:3])
                            nc.vector.tensor_scalar(z[:], z[:], mu[:, 3:4], None,
                                                    op0=ALU.mult)
                            nc.vector.tensor_tensor(z[:], z[:], lng_bc[:], op=ALU.mult)
                            nc.vector.tensor_tensor(z[:], z[:], lnb_bc[:], op=ALU.add)
                            nc.vector.tensor_copy(T(hsh_t, ic, 512)[:, :do], z[:])
                            nc.sync.dma_start(hout_b[ic * 128:(ic + 1) * 128, :], z[:])

                    if l < 3:
                        hf_next = dr.tile([N, do], F32, tag=f"h_full{l + 1}")
                        nc.gpsimd.collective_compute("AllGather", ALU.bypass,
                                                     replica_groups=RG,
                                                     ins=[hout_b.opt()],
                                                     outs=[hf_next.opt()])
                        h_full.append(hf_next)

            # ================= final: attn_he, AKED, logits =================
            with tc.tile_pool(name="fin", bufs=1) as fp_:
                hTshF = fp_.tile([128, NSH], F32, tag="hTshF")
                for ic in range(3):
                    transpose_to(hTshF[:, ic * 128:ic * 128 + 128],
                                 T(hsh_t, ic, 512)[:, :128], 128, 128)
                heoT_t = fp_.tile([128, 12 * 128], F32, tag="heoT")
                for kb in range(12):
                    transpose_to(T(heoT_t, kb, 128), T(heo_full_t, kb, 256)[:, :128],
                                 128, 128)
                ei_b = dr.tile([1, M], F32, tag="ei_b")
                ei_f = dr.tile([1, M], F32, tag="ei_f")
                pei = [pacc([1, 512]) for _ in range(3)]
                for ic in range(3):
                    psc = psc_tile()
                    for sl in range(3):
                        nc.tensor.matmul(psc[:, sl * 512:sl * 512 + 512],
                                         hTshF[:, ic * 128:ic * 128 + 128],
                                         heoT_t[:, sl * 512:sl * 512 + 512],
                                         start=True, stop=True)
                    mx = fp_.tile([128, 4], F32, tag="mx", bufs=2)
                    nc.vector.tensor_reduce(mx[:, 0:1], psc[:, 0:1536], axis=AX, op=ALU.max)
                    nc.vector.tensor_scalar(mx[:, 1:2], mx[:, 0:1], -1.0, None, op0=ALU.mult)
                    eh = fp_.tile([128, 1536], F32, tag="eh", bufs=2)
                    nc.scalar.activation(eh[:], psc[:, 0:1536], ACTF.Exp,
                                         bias=mx[:, 1:2], accum_out=mx[:, 2:3])
                    nc.vector.reciprocal(mx[:, 3:4], mx[:, 2:3])
                    nc.vector.tensor_scalar(eh[:], eh[:], mx[:, 3:4], None, op0=ALU.mult)
                    for sl in range(3):
                        nc.tensor.matmul(pei[sl][:1, :], ones_col[:],
                                         eh[:, sl * 512:sl * 512 + 512],
                                         start=(ic == 0), stop=(ic == 2),
                                         skip_group_check=True)
                ei_sb = fp_.tile([1, M], F32, tag="ei_sb")
                for sl in range(3):
                    nc.vector.tensor_scalar(ei_sb[:, sl * 512:sl * 512 + 512],
                                            pei[sl][:1, :], 1.0 / N, None, op0=ALU.mult)
                nc.sync.dma_start(ei_b[:], ei_sb[:])
                nc.gpsimd.collective_compute("AllReduce", ALU.add, replica_groups=RG,
                                             ins=[ei_b.opt()], outs=[ei_f.opt()])
                ei2 = fp_.tile([1, M], F32, tag="ei2")
                nc.sync.dma_start(ei2[:], ei_f[:])
                dm_sb = fp_.tile([1, M], F32, tag="dm_sb")
                nc.vector.tensor_scalar(dm_sb[:], ei2[:], 0.0, None, op0=ALU.is_gt)
                nc.sync.dma_start(out_dm[:], dm_sb[:])
                dmbc = fp_.tile([128, M], F32, tag="dmbc")
                for sl in range(3):
                    pb = pacc()
                    nc.tensor.matmul(pb[:, :512], ones_row[:],
                                     dm_sb[:, sl * 512:sl * 512 + 512], start=True, stop=True)
                    nc.scalar.activation(dmbc[:, sl * 512:sl * 512 + 512], pb[:, :512],
                                         ACTF.Copy)
                clsb_bc = bcast_row(I["cls_b"][:], NCLS, "clsb_bc")
                for ic in range(3):
                    hr = fp_.tile([128, M], F32, tag="hr", bufs=2)
                    nc.sync.dma_start(hr[:], I["Hrows"][ic * 128:(ic + 1) * 128, :])
                    nc.vector.tensor_tensor(hr[:], hr[:], dmbc[:], op=ALU.mult)
                    nc.sync.dma_start(out_Hd[ic * 128:(ic + 1) * 128, :], hr[:])
                    pl = pacc([128, NCLS])
                    nc.tensor.matmul(pl[:], hTshF[:, ic * 128:ic * 128 + 128], clsw_t[:],
                                     start=True, stop=True)
                    lg = fp_.tile([128, NCLS], F32, tag="lg", bufs=2)
                    nc.vector.tensor_tensor(lg[:], pl[:], clsb_bc[:], op=ALU.add)
                    nc.sync.dma_start(out_logits[ic * 128:(ic + 1) * 128, :], lg[:])

    nc.compile()
    return nc


_NC = None

def _get_nc():
    global _NC
    if _NC is None:
        _NC = build_program()
    return _NC


def _prep_inputs(x, H, U_r, params):
    x = np.asarray(x, np.float32); H = np.asarray(H, np.float32)
    U_r = np.asarray(U_r, np.float32)
    P = params
    HT = np.ascontiguousarray(H.T)
    common = {}
    common["HT_bf"] = HT.astype(ml_dtypes.bfloat16)
    inw = np.zeros((FP, D0), np.float32); inw[:F] = np.asarray(P["in_w"], np.float32)
    common["inw"] = inw
    common["inb"] = np.asarray(P["in_b"], np.float32).reshape(1, D0)
    for l, (di, do) in enumerate(LAYERS):
        hd = do // HEADS
        s = hd + 1
        W = 8 * s
        Lp = P["layers"][l]
        Wq = np.asarray(Lp["Wq"], np.float32); Wk = np.asarray(Lp["Wk"], np.float32)
        Wv = np.asarray(Lp["Wv"], np.float32)
        Wqp = np.zeros((di, 256), np.float32); Wkp = np.zeros((di, 256), np.float32)
        Wva = np.zeros((di, W), np.float32)
        for h in range(HEADS):
            Wqp[:, 32 * h:32 * h + hd] = Wq[:, hd * h:hd * (h + 1)]
            Wkp[:, 32 * h:32 * h + hd] = Wk[:, hd * h:hd * (h + 1)]
            Wva[:, s * h:s * h + hd] = Wv[:, hd * h:hd * (h + 1)]
        common[f"Wqp{l}"] = Wqp; common[f"Wkp{l}"] = Wkp; common[f"Wva{l}"] = Wva
        Wg3 = np.zeros((256, 24), np.float32)
        for gi, nm in enumerate(["Wgn", "Wge", "Wgs"]):
            Wg = np.asarray(Lp[nm], np.float32)  # [do, 8]
            for h in range(HEADS):
                Wg3[32 * h:32 * h + hd, gi * 8:gi * 8 + 8] = Wg[hd * h:hd * (h + 1), :]
        common[f"Wg3{l}"] = Wg3
        common[f"bg3{l}"] = np.concatenate(
            [np.asarray(Lp[nm], np.float32) for nm in ["bgn", "bge", "bgs"]]).reshape(1, 24)
        common[f"Wsp{l}"] = np.asarray(Lp["Wspec"], np.float32)
        common[f"Wo{l}"] = np.asarray(Lp["Wout"], np.float32)
        common[f"bo{l}"] = np.asarray(Lp["bout"], np.float32).reshape(1, do)
        common[f"lng{l}"] = np.asarray(Lp["ln_g"], np.float32).reshape(1, do)
        common[f"lnb{l}"] = np.asarray(Lp["ln_b"], np.float32).reshape(1, do)
    common["cls_w"] = np.asarray(P["cls_w"], np.float32)
    common["cls_b"] = np.asarray(P["cls_b"], np.float32).reshape(1, NCLS)

    in_maps = []
    for c in range(C):
        r0, r1 = c * NSH, (c + 1) * NSH
        e0, e1 = c * MSH, (c + 1) * MSH
        m = dict(common)
        xTc = np.zeros((FP, NSH), np.float32)
        xTc[:F] = x[r0:r1].T
        m["xT"] = xTc
        m["HTs"] = np.ascontiguousarray(HT[:, r0:r1])
        m["Hs"] = np.ascontiguousarray(H[:, e0:e1])
        m["Hrows"] = np.ascontiguousarray(H[r0:r1])
        m["U_rTs"] = np.ascontiguousarray(np.asarray(U_r, np.float32)[r0:r1].T)
        in_maps.append(m)
    return in_maps


def kernel(x, H, U_r, params):
    nc = _get_nc()
    in_maps = _prep_inputs(x, H, U_r, params)
    res = run_bass_kernel_spmd(nc, in_maps, core_ids=list(range(C)))
    logits = np.concatenate([res.results[c]["logits_sh"] for c in range(C)], axis=0)
    Hd = np.concatenate([res.results[c]["Hd_sh"] for c in range(C)], axis=0)
    dm = res.results[0]["dmask"].reshape(M)
    return (logits, Hd, dm)
